# revision 33
# baseline (speedup 1.0000x reference)
"""AttentionDCA energy kernel for 8 Trainium2 NeuronCores (fp8 edition).

Math: with one-hot E_b in {0,1}^{L x 21} for sequence x[b],
    energy[b] = -sum_h <E_b^T A_h E_b, V_h>_F
where A_h = softmax(Q_h K_h^T / d_k).  Everything becomes PE matmuls:

  per core (4 heads, H sharded over 8 cores):
    S_T[j,i]   = K_h Q_h^T           fp8 DoubleRow (K=128 as 64x2)
    expS       = exp(S_T / d_k)      ScalarE, PSUM->SBUF fp8
    R'[i,col]  = sum_j expS[j,i] EA[j,col]   fp8 DoubleRow (K=256 j-pairs;
                 EA = one-hot for all 64 b + ones column -> row sums r)
    R          = R' * (1/r[i])       eviction w/ per-partition scale, bf16,
                 split across ScalarE / VectorE
    C          = E_g^T R_g           PE bf16 moving (6 b block-diag, M=128)
    S[p,g]     = sum(C * VV)         DVE fused scalar_tensor_tensor

Host shards heads (4/core), stages fp8/bf16 inputs, sums the 8 cores'
[128, 11] partials and does the tiny cross-partition reduction.
"""

import numpy as np
import ml_dtypes

# Problem constants (hardcoded per contract)
B, L, H, D, NAA = 64, 512, 32, 128, 21
NCORES = 8
HPC = H // NCORES            # heads per core = 4
JB = L // 128                # 4 position blocks
JBP = JB // 2                # 2 double-row pairs of 256
BG = 6                       # b's per group
NG = (B + BG - 1) // BG      # 11 groups
BPAD = BG * NG               # 66 padded batch
NE = BPAD * NAA              # 1386 one-hot columns
NEA = 1392                   # padded column count
CHK = 464                    # columns per PSUM-bank chunk (chunks 0,1)
ONES_COL = B * NAA           # 1344: column of ones -> softmax row sums
CHK2 = 424                   # chunk 2 covers cols 928..1352
RCOL = ONES_COL - 2 * CHK    # 416: ones-column index inside chunk 2
MP = BG * NAA                # 126 used output partitions of mat2
NQ = HPC * MP                # 504 output free size of mat2

_NC = None


def _build_nc():
    import concourse.bacc as bacc
    import concourse.tile as tile
    from concourse import mybir

    f32 = mybir.dt.float32
    bf16 = mybir.dt.bfloat16
    f8 = mybir.dt.float8e4
    AF = mybir.ActivationFunctionType
    DR = mybir.MatmulPerfMode.DoubleRow
    ALU = mybir.AluOpType

    nc = bacc.Bacc("TRN2", target_bir_lowering=False, debug=False)

    # qk8[p, t, qk, l]: per-head Q/K^T in fp8, contraction D=128 split 64x2
    qk_d = [nc.dram_tensor(f"qk{h}", [64, 2, 2, L], f8, kind="ExternalInput")
            for h in range(HPC)]
    # eb8[p, bp, t, col]: one-hot EA at position bp*256 + t*128 + p
    eb_d = nc.dram_tensor("eb", [128, JBP, 2, NEA], f8, kind="ExternalInput")
    vv_d = nc.dram_tensor("vv", [128, NQ], bf16, kind="ExternalInput")
    out_d = nc.dram_tensor("energy", [128, NG], f32, kind="ExternalOutput")

    with tile.TileContext(nc) as tc:
        with (
            tc.tile_pool(name="const", bufs=1) as cpool,
            tc.tile_pool(name="exps", bufs=HPC) as xpool,
            tc.tile_pool(name="rall", bufs=1) as rpool,
            tc.tile_pool(name="small", bufs=8) as spool,
            tc.tile_pool(name="psum", bufs=8, space="PSUM") as pp,
        ):
            qk_sb = [cpool.tile([64, 2, 2, L], f8, tag=f"qk{h}",
                                name=f"qk{h}sb") for h in range(HPC)]
            eb_sb = cpool.tile([128, JBP, 2, NEA], f8, tag="eb")
            vv_sb = cpool.tile([128, NQ], bf16, tag="vv")
            s_sb = cpool.tile([128, NG], f32, tag="ssb")

            # order by first-use time: heads 0-1 gate scores, eb gates
            # mat1(0), heads 2-3 aren't needed until mat1(0) ends
            nc.sync.dma_start(out=qk_sb[0][:], in_=qk_d[0][:])
            nc.sync.dma_start(out=qk_sb[1][:], in_=qk_d[1][:])
            nc.sync.dma_start(out=eb_sb[:], in_=eb_d[:])
            nc.sync.dma_start(out=qk_sb[2][:], in_=qk_d[2][:])
            nc.sync.dma_start(out=qk_sb[3][:], in_=qk_d[3][:])
            nc.sync.dma_start(out=vv_sb[:], in_=vv_d[:])

            r_sb = rpool.tile([128, JB, HPC, NEA], bf16, tag="r")
            # cols [1352:1392) are never written by evictions; zero them so
            # group 10's (discarded pad-b) matmul reads no NaN garbage
            nc.vector.memset(r_sb[:, :, :, 2 * CHK + CHK2:], 0.0)

            exps = []

            def scores(h):
                # one ex tile per j-block pair so mat1's first accumulation
                # pass starts after just two exps
                ex = [xpool.tile([128, 2, L], f8, tag=f"ex{p}",
                                 name=f"ex{p}") for p in range(JBP)]
                exps.append(ex)
                qq = qk_sb[h]
                for jb in range(JB):
                    ps = pp.tile([128, L], f32, tag="ps")
                    nc.tensor.matmul(
                        ps[:],
                        qq[:, :, 1, jb * 128:(jb + 1) * 128],
                        qq[:, :, 0, :],
                        start=True,
                        stop=True,
                        perf_mode=DR,
                    )
                    nc.scalar.activation(
                        ex[jb // 2][:, jb % 2, :], ps[:], AF.Exp,
                        scale=1.0 / D,
                    )

            scores(0)
            scores(1)

            for h in range(HPC):
                ex = exps[h]
                for ib in range(JB):
                    # 3 chunk accumulators in separate 1-bank PSUM tiles so
                    # each frees as soon as its own eviction lands
                    prs = [pp.tile([128, L], f32, tag="ps", name=f"pr{ck}")
                           for ck in range(3)]
                    for jbp in range(JBP):
                        lhs = ex[jbp][:, :, ib * 128:(ib + 1) * 128]
                        # chunk 2 (ones column) finishes first so the
                        # reciprocal overlaps the remaining streams
                        for ck in (2, 0, 1):
                            w = CHK2 if ck == 2 else CHK
                            nc.tensor.matmul(
                                prs[ck][:, :w],
                                lhs,
                                eb_sb[:, jbp, :, ck * CHK:ck * CHK + w],
                                start=(jbp == 0),
                                stop=(jbp == JBP - 1),
                                perf_mode=DR,
                            )
                    rcp = spool.tile([128, 1], f32, tag="rcp")
                    nc.vector.reciprocal(rcp[:], prs[2][:, RCOL:RCOL + 1])
                    # evict with 1/r scale (PSUM f32 -> SBUF bf16),
                    # balanced across ScalarE and VectorE (GpSimd cannot
                    # read PSUM; DMA cannot read PSUM either)
                    nc.scalar.mul(
                        r_sb[:, ib, h, :CHK],
                        prs[0][:, :CHK],
                        rcp[:],
                    )
                    nc.vector.tensor_scalar_mul(
                        r_sb[:, ib, h, CHK:2 * CHK],
                        prs[1][:, :CHK],
                        rcp[:],
                    )
                    if ib % 2 == 0:
                        nc.scalar.mul(
                            r_sb[:, ib, h, 2 * CHK:2 * CHK + CHK2],
                            prs[2][:, :CHK2],
                            rcp[:],
                        )
                    else:
                        nc.vector.tensor_scalar_mul(
                            r_sb[:, ib, h, 2 * CHK:2 * CHK + CHK2],
                            prs[2][:, :CHK2],
                            rcp[:],
                        )
                if h + 2 < HPC:
                    scores(h + 2)

            # Phase 3: C = E^T R (block-diagonal over 6 b's), V-weighted
            # reduce fused on DVE
            for g in range(NG):
                pc = pp.tile([128, NQ], f32, tag="ps")
                for ib in range(JB):
                    nc.tensor.matmul(
                        pc[:],
                        eb_sb[:, ib // 2, ib % 2, g * MP:g * MP + 128],
                        r_sb[:, ib, :, g * MP:(g + 1) * MP],
                        start=(ib == 0),
                        stop=(ib == JB - 1),
                    )
                scr = spool.tile([128, NQ], f32, tag="scr")
                nc.vector.scalar_tensor_tensor(
                    scr[:], pc[:], 1.0, vv_sb[:],
                    ALU.mult, ALU.mult,
                    accum_out=s_sb[:, g:g + 1],
                )

            # Phase 4: per-partition partials go straight to the host,
            # which does the tiny cross-partition sum; first half ships
            # while the last groups still compute
            nc.sync.dma_start(out=out_d[:, :6], in_=s_sb[:, :6])
            nc.sync.dma_start(out=out_d[:, 6:], in_=s_sb[:, 6:])

    nc.compile()
    return nc


def _get_nc():
    global _NC
    if _NC is None:
        _NC = _build_nc()
    return _NC


def _stage_inputs(x, Q, K, V):
    """Host-side sharding/staging. Returns in_maps for the 8 cores."""
    f8 = ml_dtypes.float8_e4m3
    bf16 = ml_dtypes.bfloat16
    x = np.asarray(x)
    Q = np.asarray(Q, dtype=np.float32)
    K = np.asarray(K, dtype=np.float32)
    V = np.asarray(V, dtype=np.float32)

    # One-hot EA [L, NEA] (+ ones column), replicated to all cores
    onehot = (x[:, :, None] == np.arange(NAA, dtype=x.dtype)[None, None, :])
    ea = np.zeros((L, NEA), dtype=np.float32)
    ea[:, : B * NAA] = onehot.transpose(1, 0, 2).reshape(L, B * NAA)
    ea[:, ONES_COL] = 1.0  # col 1344
    # [pos, col] -> [p, bp, t, col] with pos = bp*256 + t*128 + p
    eb_host = np.ascontiguousarray(
        ea.reshape(JBP, 2, 128, NEA).transpose(2, 0, 1, 3)
    ).astype(f8)

    in_maps = []
    for c in range(NCORES):
        hs = slice(c * HPC, (c + 1) * HPC)
        # [h, L, D] -> [D, h, L] -> [t(2), 64, h, L] -> [64, t, h, L]
        qt = Q[hs].transpose(2, 0, 1).reshape(2, 64, HPC, L).transpose(1, 0, 2, 3)
        kt = K[hs].transpose(2, 0, 1).reshape(2, 64, HPC, L).transpose(1, 0, 2, 3)
        # stack q/k -> [64, t, qk, h, L]
        qk = np.stack([qt, kt], axis=2)
        vv = np.zeros((128, NQ), dtype=np.float32)
        vc = V[hs]
        for h in range(HPC):
            for bl in range(BG):
                vv[bl * NAA:(bl + 1) * NAA,
                   h * MP + bl * NAA: h * MP + (bl + 1) * NAA] = -vc[h]
        im = {"eb": eb_host, "vv": vv.astype(bf16)}
        for h in range(HPC):
            im[f"qk{h}"] = np.ascontiguousarray(qk[:, :, :, h]).astype(f8)
        in_maps.append(im)
    return in_maps


def _run(x, Q, K, V, trace=False):
    from concourse.bass_utils import run_bass_kernel_spmd

    nc = _get_nc()
    in_maps = _stage_inputs(x, Q, K, V)
    res = run_bass_kernel_spmd(nc, in_maps, list(range(NCORES)), trace=trace)

    total = np.zeros((128, NG), dtype=np.float64)
    for r in res.results:
        total += r["energy"].astype(np.float64)
    # host-side cross-partition sum: partition p = bl*21 + a for bl < 6
    blk = total[:BG * NAA].reshape(BG, NAA, NG).sum(axis=1)  # [BG, NG]
    bidx = np.arange(B)
    energy = blk[bidx % BG, bidx // BG].astype(np.float32)
    return energy, res


def kernel(x, Q, K, V):
    return _run(x, Q, K, V)[0]


# revision 34
# speedup vs baseline: 1.1705x; 1.1705x over previous
"""AttentionDCA energy kernel for 8 Trainium2 NeuronCores (fp8 edition).

Math: with one-hot E_b in {0,1}^{L x 21} for sequence x[b],
    energy[b] = -sum_h <E_b^T A_h E_b, V_h>_F
where A_h = softmax(Q_h K_h^T / d_k).  Everything becomes PE matmuls:

  per core (4 heads, H sharded over 8 cores):
    S_T[j,i]   = K_h Q_h^T           fp8 DoubleRow (K=128 as 64x2)
    expS       = exp(S_T / d_k)      ScalarE, PSUM->SBUF fp8
    R'[i,col]  = sum_j expS[j,i] EA[j,col]   fp8 DoubleRow (K=256 j-pairs;
                 EA = one-hot for all 64 b + ones column -> row sums r)
    R          = R' * (1/r[i])       eviction w/ per-partition scale, bf16,
                 split across ScalarE / VectorE
    C          = E_g^T R_g           PE bf16 moving (6 b block-diag, M=128)
    S[p,g]     = sum(C * VV)         DVE fused scalar_tensor_tensor

Host shards heads (4/core), stages fp8/bf16 inputs, sums the 8 cores'
[128, 11] partials and does the tiny cross-partition reduction.
"""

import numpy as np
import ml_dtypes

# Problem constants (hardcoded per contract)
B, L, H, D, NAA = 64, 512, 32, 128, 21
NCORES = 8
HPC = H // NCORES            # heads per core = 4
JB = L // 128                # 4 position blocks
JBP = JB // 2                # 2 double-row pairs of 256
BG = 6                       # b's per group
NG = (B + BG - 1) // BG      # 11 groups
BPAD = BG * NG               # 66 padded batch
NE = BPAD * NAA              # 1386 one-hot columns
NEA = 1392                   # padded column count
CHK = 464                    # columns per PSUM-bank chunk (chunks 0,1)
ONES_COL = B * NAA           # 1344: column of ones -> softmax row sums
CHK2 = 424                   # chunk 2 covers cols 928..1352
RCOL = ONES_COL - 2 * CHK    # 416: ones-column index inside chunk 2
MP = BG * NAA                # 126 used output partitions of mat2
NQ = HPC * MP                # 504 output free size of mat2

_NC = None


def _build_nc():
    import concourse.bacc as bacc
    import concourse.tile as tile
    from concourse import mybir

    f32 = mybir.dt.float32
    bf16 = mybir.dt.bfloat16
    f8 = mybir.dt.float8e4
    AF = mybir.ActivationFunctionType
    DR = mybir.MatmulPerfMode.DoubleRow
    ALU = mybir.AluOpType

    nc = bacc.Bacc("TRN2", target_bir_lowering=False, debug=False)

    # qk8[p, t, qk, l]: per-head Q/K^T in fp8, contraction D=128 split 64x2
    qk_d = [nc.dram_tensor(f"qk{h}", [64, 2, 2, L], f8, kind="ExternalInput")
            for h in range(HPC)]
    # eb8[p, bp, t, col]: one-hot EA at position bp*256 + t*128 + p
    eb_d = nc.dram_tensor("eb", [128, JBP, 2, NEA], f8, kind="ExternalInput")
    vv_d = nc.dram_tensor("vv", [128, NQ], bf16, kind="ExternalInput")
    out_d = nc.dram_tensor("energy", [128, NG], f32, kind="ExternalOutput")

    with tile.TileContext(nc) as tc:
        with (
            tc.tile_pool(name="const", bufs=1) as cpool,
            tc.tile_pool(name="exps", bufs=HPC) as xpool,
            tc.tile_pool(name="rall", bufs=1) as rpool,
            tc.tile_pool(name="small", bufs=8) as spool,
            tc.tile_pool(name="psum", bufs=8, space="PSUM") as pp,
        ):
            qk_sb = [cpool.tile([64, 2, 2, L], f8, tag=f"qk{h}",
                                name=f"qk{h}sb") for h in range(HPC)]
            eb_sb = cpool.tile([128, JBP, 2, NEA], f8, tag="eb")
            vv_sb = cpool.tile([128, NQ], bf16, tag="vv")
            s_sb = cpool.tile([128, NG], f32, tag="ssb")

            # order by first-use time: heads 0-1 gate scores, eb gates
            # mat1(0), heads 2-3 aren't needed until mat1(0) ends
            nc.sync.dma_start(out=qk_sb[0][:], in_=qk_d[0][:])
            nc.sync.dma_start(out=qk_sb[1][:], in_=qk_d[1][:])
            nc.sync.dma_start(out=eb_sb[:], in_=eb_d[:])
            nc.sync.dma_start(out=qk_sb[2][:], in_=qk_d[2][:])
            nc.sync.dma_start(out=qk_sb[3][:], in_=qk_d[3][:])
            nc.sync.dma_start(out=vv_sb[:], in_=vv_d[:])

            r_sb = rpool.tile([128, JB, HPC, NEA], bf16, tag="r")
            # cols [1352:1392) are never written by evictions; zero them so
            # group 10's (discarded pad-b) matmul reads no NaN garbage
            nc.vector.memset(r_sb[:, :, :, 2 * CHK + CHK2:], 0.0)

            # Wake the PE sequencer during the fixed runtime head: without
            # this the first real matmul dispatches ~1.8us after its input
            # DMA lands (observed); a ~200ns dummy chain absorbs that
            warm_sb = cpool.tile([128, 48], f8, tag="warm")
            nc.gpsimd.memset(warm_sb[:], 0.0)
            pw = pp.tile([32, 48], f32, tag="ps", name="pwarm")
            for _ in range(4):
                nc.tensor.matmul(
                    pw[:], warm_sb[:, :32], warm_sb[:],
                    start=True, stop=True,
                )

            exps = []

            def scores(h):
                # one ex tile per j-block pair so mat1's first accumulation
                # pass starts after just two exps
                ex = [xpool.tile([128, 2, L], f8, tag=f"ex{p}",
                                 name=f"ex{p}") for p in range(JBP)]
                exps.append(ex)
                qq = qk_sb[h]
                for jb in range(JB):
                    ps = pp.tile([128, L], f32, tag="ps")
                    nc.tensor.matmul(
                        ps[:],
                        qq[:, :, 1, jb * 128:(jb + 1) * 128],
                        qq[:, :, 0, :],
                        start=True,
                        stop=True,
                        perf_mode=DR,
                    )
                    nc.scalar.activation(
                        ex[jb // 2][:, jb % 2, :], ps[:], AF.Exp,
                        scale=1.0 / D,
                    )

            scores(0)
            scores(1)

            for h in range(HPC):
                ex = exps[h]
                for ib in range(JB):
                    # 3 chunk accumulators in separate 1-bank PSUM tiles so
                    # each frees as soon as its own eviction lands
                    prs = [pp.tile([128, L], f32, tag="ps", name=f"pr{ck}")
                           for ck in range(3)]
                    for jbp in range(JBP):
                        lhs = ex[jbp][:, :, ib * 128:(ib + 1) * 128]
                        # chunk 2 (ones column) finishes first so the
                        # reciprocal overlaps the remaining streams
                        for ck in (2, 0, 1):
                            w = CHK2 if ck == 2 else CHK
                            nc.tensor.matmul(
                                prs[ck][:, :w],
                                lhs,
                                eb_sb[:, jbp, :, ck * CHK:ck * CHK + w],
                                start=(jbp == 0),
                                stop=(jbp == JBP - 1),
                                perf_mode=DR,
                            )
                    rcp = spool.tile([128, 1], f32, tag="rcp")
                    nc.vector.reciprocal(rcp[:], prs[2][:, RCOL:RCOL + 1])
                    # evict with 1/r scale (PSUM f32 -> SBUF bf16),
                    # balanced across ScalarE and VectorE (GpSimd cannot
                    # read PSUM; DMA cannot read PSUM either)
                    nc.scalar.mul(
                        r_sb[:, ib, h, :CHK],
                        prs[0][:, :CHK],
                        rcp[:],
                    )
                    nc.vector.tensor_scalar_mul(
                        r_sb[:, ib, h, CHK:2 * CHK],
                        prs[1][:, :CHK],
                        rcp[:],
                    )
                    if ib % 2 == 0:
                        nc.scalar.mul(
                            r_sb[:, ib, h, 2 * CHK:2 * CHK + CHK2],
                            prs[2][:, :CHK2],
                            rcp[:],
                        )
                    else:
                        nc.vector.tensor_scalar_mul(
                            r_sb[:, ib, h, 2 * CHK:2 * CHK + CHK2],
                            prs[2][:, :CHK2],
                            rcp[:],
                        )
                if h + 2 < HPC:
                    scores(h + 2)

            # Phase 3: C = E^T R (block-diagonal over 6 b's), V-weighted
            # reduce fused on DVE
            for g in range(NG):
                pc = pp.tile([128, NQ], f32, tag="ps")
                for ib in range(JB):
                    nc.tensor.matmul(
                        pc[:],
                        eb_sb[:, ib // 2, ib % 2, g * MP:g * MP + 128],
                        r_sb[:, ib, :, g * MP:(g + 1) * MP],
                        start=(ib == 0),
                        stop=(ib == JB - 1),
                    )
                scr = spool.tile([128, NQ], f32, tag="scr")
                nc.vector.scalar_tensor_tensor(
                    scr[:], pc[:], 1.0, vv_sb[:],
                    ALU.mult, ALU.mult,
                    accum_out=s_sb[:, g:g + 1],
                )

            # Phase 4: per-partition partials go straight to the host,
            # which does the tiny cross-partition sum; first half ships
            # while the last groups still compute
            nc.sync.dma_start(out=out_d[:, :6], in_=s_sb[:, :6])
            nc.sync.dma_start(out=out_d[:, 6:], in_=s_sb[:, 6:])

    nc.compile()
    return nc


def _get_nc():
    global _NC
    if _NC is None:
        _NC = _build_nc()
    return _NC


def _stage_inputs(x, Q, K, V):
    """Host-side sharding/staging. Returns in_maps for the 8 cores."""
    f8 = ml_dtypes.float8_e4m3
    bf16 = ml_dtypes.bfloat16
    x = np.asarray(x)
    Q = np.asarray(Q, dtype=np.float32)
    K = np.asarray(K, dtype=np.float32)
    V = np.asarray(V, dtype=np.float32)

    # One-hot EA [L, NEA] (+ ones column), replicated to all cores
    onehot = (x[:, :, None] == np.arange(NAA, dtype=x.dtype)[None, None, :])
    ea = np.zeros((L, NEA), dtype=np.float32)
    ea[:, : B * NAA] = onehot.transpose(1, 0, 2).reshape(L, B * NAA)
    ea[:, ONES_COL] = 1.0  # col 1344
    # [pos, col] -> [p, bp, t, col] with pos = bp*256 + t*128 + p
    eb_host = np.ascontiguousarray(
        ea.reshape(JBP, 2, 128, NEA).transpose(2, 0, 1, 3)
    ).astype(f8)

    in_maps = []
    for c in range(NCORES):
        hs = slice(c * HPC, (c + 1) * HPC)
        # [h, L, D] -> [D, h, L] -> [t(2), 64, h, L] -> [64, t, h, L]
        qt = Q[hs].transpose(2, 0, 1).reshape(2, 64, HPC, L).transpose(1, 0, 2, 3)
        kt = K[hs].transpose(2, 0, 1).reshape(2, 64, HPC, L).transpose(1, 0, 2, 3)
        # stack q/k -> [64, t, qk, h, L]
        qk = np.stack([qt, kt], axis=2)
        vv = np.zeros((128, NQ), dtype=np.float32)
        vc = V[hs]
        for h in range(HPC):
            for bl in range(BG):
                vv[bl * NAA:(bl + 1) * NAA,
                   h * MP + bl * NAA: h * MP + (bl + 1) * NAA] = -vc[h]
        im = {"eb": eb_host, "vv": vv.astype(bf16)}
        for h in range(HPC):
            im[f"qk{h}"] = np.ascontiguousarray(qk[:, :, :, h]).astype(f8)
        in_maps.append(im)
    return in_maps


def _run(x, Q, K, V, trace=False):
    from concourse.bass_utils import run_bass_kernel_spmd

    nc = _get_nc()
    in_maps = _stage_inputs(x, Q, K, V)
    res = run_bass_kernel_spmd(nc, in_maps, list(range(NCORES)), trace=trace)

    total = np.zeros((128, NG), dtype=np.float64)
    for r in res.results:
        total += r["energy"].astype(np.float64)
    # host-side cross-partition sum: partition p = bl*21 + a for bl < 6
    blk = total[:BG * NAA].reshape(BG, NAA, NG).sum(axis=1)  # [BG, NG]
    bidx = np.arange(B)
    energy = blk[bidx % BG, bidx // BG].astype(np.float32)
    return energy, res


def kernel(x, Q, K, V):
    return _run(x, Q, K, V)[0]
